# revision 1
# baseline (speedup 1.0000x reference)
"""Trainium2 Bass kernel for Bahdanau-style attention (nn_Attention_35777077575919).

Reference computation (per batch b):
    key    = relu(enc[b] @ Wk.T + bk)          # [T, A]
    query  = relu(dec[b] @ Wq.T + bq)          # [A]
    energy = key @ query                       # [T]
    attn   = softmax(energy) * mask[b]; attn /= attn.sum()
    ctx    = attn @ enc[b]                     # [E]
    returns (attn [B,1,T], ctx [B,E])

Shapes: B=32, T=2048, E=2048 (enc dim), H=1024, A=512.

Strategy (8 NeuronCores, data-parallel over batch, 4 batches/core):
  - Host pre-transposes enc -> encT [b, E, T] so the E-contraction of the key
    matmul has E on SBUF partitions with contiguous DMA rows.
  - Key matmul in fp32r (full-rate fp32 on the PE): for each 512-wide t-chunk,
    psum[a128, t512] += WkT[e128, a128].T @ encT[e128, t512] over 16 e-chunks.
  - Relu+bias fused on ScalarE writing fp32r keyT tiles.
  - Energy via PE with a replicated-query stationary ([a128, 128] all-same
    columns) so the energy row lands replicated across all 128 partitions,
    making the whole softmax a full-width SIMD computation.
  - Online (flash-style) softmax over t-chunks: running max m, running sum Z,
    and the context accumulator rescaled by exp(m_old - m_new) each chunk.
    Mask is folded in as energy += ln(mask).
  - Context (attn-weighted sum of enc rows) via DVE affine_mul_reduce directly
    on the [e,t]-layout tiles already in SBUF: ctx_partial[e] += sum_t
    encT[e,t] * w[t]. No second pass over enc, no transposes.
"""

import sys

if "/opt/trn_rl_repo" not in sys.path:
    sys.path.insert(0, "/opt/trn_rl_repo")

import numpy as np

B, T, E, A, H = 32, 2048, 2048, 512, 1024
NCORES = 8
BPC = B // NCORES  # batches per core
P = 128
TC = 512           # t-chunk width
NTC = T // TC      # 4 t-chunks
EK = E // P        # 16 e-chunks (contraction)
AC = A // P        # 4 a-chunks
HK = H // P        # 8 h-chunks

_CACHE = {}


def _build_nc():
    import concourse.mybir as mybir
    import concourse.tile as tile
    from concourse import bacc

    F32 = mybir.dt.float32
    F32R = mybir.dt.float32r
    AF = mybir.ActivationFunctionType
    ALU = mybir.AluOpType
    AX = mybir.AxisListType

    nc = bacc.Bacc()
    encT = nc.declare_dram_parameter("enct", [BPC, E, T], F32, isOutput=False)
    wkT = nc.declare_dram_parameter("wkt", [E, A], F32, isOutput=False)
    wqT = nc.declare_dram_parameter("wqt", [H, A], F32, isOutput=False)
    dec = nc.declare_dram_parameter("dec", [H, BPC], F32, isOutput=False)
    mask = nc.declare_dram_parameter("mask", [BPC, T], F32, isOutput=False)
    bkp = nc.declare_dram_parameter("bk", [A], F32, isOutput=False)
    bqp = nc.declare_dram_parameter("bq", [A], F32, isOutput=False)
    attn_out = nc.declare_dram_parameter("attn", [BPC, T], F32, isOutput=True)
    ctx_out = nc.declare_dram_parameter("ctx", [BPC, E], F32, isOutput=True)

    with tile.TileContext(nc) as tc:
        with (
            tc.tile_pool(name="const", bufs=1) as const,
            tc.tile_pool(name="enc", bufs=2) as encp,
            tc.tile_pool(name="key", bufs=2) as keyp,
            tc.tile_pool(name="big", bufs=2) as bigp,
            tc.tile_pool(name="small", bufs=3) as smallp,
            tc.tile_pool(name="chain", bufs=4) as chainp,
            tc.tile_pool(name="psk", bufs=4, space="PSUM") as pskp,
            tc.tile_pool(name="pse", bufs=2, space="PSUM") as psep,
            tc.tile_pool(name="psq", bufs=1, space="PSUM") as psqp,
        ):
            # ---- constants -------------------------------------------------
            wk_sb = const.tile([P, EK, A], F32R)
            nc.sync.dma_start(
                wk_sb[:], wkT.rearrange("(ek p) a -> p ek a", p=P).bitcast(F32R)
            )
            wq_sb = const.tile([P, HK, A], F32R)
            nc.sync.dma_start(
                wq_sb[:], wqT.rearrange("(hk p) a -> p hk a", p=P).bitcast(F32R)
            )
            dec_sb = const.tile([P, HK, BPC], F32R)
            nc.sync.dma_start(
                dec_sb[:], dec.rearrange("(hk p) b -> p hk b", p=P).bitcast(F32R)
            )
            bk_sb = const.tile([P, AC], F32)
            nc.sync.dma_start(bk_sb[:], bkp.rearrange("(ac p) -> p ac", p=P))
            bq_sb = const.tile([P, AC], F32)
            nc.sync.dma_start(bq_sb[:], bqp.rearrange("(ac p) -> p ac", p=P))

            # ---- query: q = relu(WqT.T @ dec + bq), all 4 batches at once --
            q_sb = const.tile([P, AC * BPC], F32R)  # [(ac, b)] on free dim
            for ac in range(AC):
                psq = psqp.tile([P, BPC], F32, tag="psq")
                for hk in range(HK):
                    nc.tensor.matmul(
                        psq[:],
                        wq_sb[:, hk, ac * P : (ac + 1) * P],
                        dec_sb[:, hk, :],
                        start=(hk == 0),
                        stop=(hk == HK - 1),
                    )
                nc.scalar.activation(
                    q_sb[:, ac * BPC : (ac + 1) * BPC],
                    psq[:],
                    AF.Relu,
                    bias=bq_sb[:, ac : ac + 1],
                    scale=1.0,
                )
            # replicate each q column across the 128-wide stationary dim so the
            # energy matmul output is replicated across partitions
            q_rep = const.tile([P, AC * BPC, P], F32R)
            for i in range(AC * BPC):
                nc.vector.tensor_copy(
                    q_rep[:, i, :], q_sb[:, i : i + 1].to_broadcast((P, P))
                )

            # ---- main loop -------------------------------------------------
            for b in range(BPC):
                logmask = bigp.tile([P, T], F32, tag="logmask")
                nc.sync.dma_start(logmask[:], mask[b : b + 1, :].to_broadcast((P, T)))
                nc.scalar.activation(logmask[:], logmask[:], AF.Ln, bias=0.0, scale=1.0)

                attn_rep = bigp.tile([P, T], F32, tag="attnrep")
                m_hist = smallp.tile([P, NTC], F32, tag="mhist")

                m_run = chainp.tile([P, 1], F32, tag="m")
                nc.vector.memset(m_run[:], -1.0e30)
                z_run = chainp.tile([P, 1], F32, tag="z")
                nc.vector.memset(z_run[:], 0.0)
                ctx_run = chainp.tile([P, EK], F32, tag="ctx")
                nc.vector.memset(ctx_run[:], 0.0)

                for tci in range(NTC):
                    ts = slice(tci * TC, (tci + 1) * TC)
                    enc_t = encp.tile([P, EK, TC], F32R, tag="enc")
                    nc.sync.dma_start(
                        enc_t[:],
                        encT[b, :, ts]
                        .rearrange("(ek p) t -> p ek t", p=P)
                        .bitcast(F32R),
                    )
                    # keyT[a, t] = relu(sum_e WkT[e,a] * encT[e,t] + bk[a])
                    key_sb = keyp.tile([P, AC, TC], F32R, tag="key")
                    psks = []
                    for ac in range(AC):
                        psk = pskp.tile([P, TC], F32, tag="psk")
                        psks.append(psk)
                        for ek in range(EK):
                            nc.tensor.matmul(
                                psk[:],
                                wk_sb[:, ek, ac * P : (ac + 1) * P],
                                enc_t[:, ek, :],
                                start=(ek == 0),
                                stop=(ek == EK - 1),
                            )
                    for ac in range(AC):
                        nc.scalar.activation(
                            key_sb[:, ac, :],
                            psks[ac][:],
                            AF.Relu,
                            bias=bk_sb[:, ac : ac + 1],
                            scale=1.0,
                        )
                    # energy (replicated across partitions)
                    pse = psep.tile([P, TC], F32, tag="pse")
                    for ac in range(AC):
                        nc.tensor.matmul(
                            pse[:],
                            q_rep[:, ac * BPC + b, :],
                            key_sb[:, ac, :],
                            start=(ac == 0),
                            stop=(ac == AC - 1),
                        )
                    # e_adj = energy + ln(mask)
                    e_adj = smallp.tile([P, TC], F32, tag="eadj")
                    nc.vector.tensor_add(e_adj[:], pse[:], logmask[:, ts])
                    # flash-softmax update
                    cmax = smallp.tile([P, 1], F32, tag="cmax")
                    nc.vector.reduce_max(cmax[:], e_adj[:], axis=AX.X)
                    m_new = chainp.tile([P, 1], F32, tag="m")
                    nc.vector.tensor_max(m_new[:], m_run[:], cmax[:])
                    neg_m = smallp.tile([P, 1], F32, tag="negm")
                    nc.vector.tensor_scalar_mul(neg_m[:], m_new[:], -1.0)
                    r_fac = smallp.tile([P, 1], F32, tag="rfac")
                    nc.scalar.activation(
                        r_fac[:], m_run[:], AF.Exp, bias=neg_m[:], scale=1.0
                    )
                    csum = smallp.tile([P, 1], F32, tag="csum")
                    nc.scalar.activation(
                        attn_rep[:, ts],
                        e_adj[:],
                        AF.Exp,
                        bias=neg_m[:],
                        scale=1.0,
                        accum_out=csum[:],
                    )
                    z_new = chainp.tile([P, 1], F32, tag="z")
                    nc.vector.tensor_scalar(
                        out=z_new[:],
                        in0=z_run[:],
                        scalar1=r_fac[:],
                        scalar2=csum[:],
                        op0=ALU.mult,
                        op1=ALU.add,
                    )
                    nc.vector.tensor_copy(m_hist[:, tci : tci + 1], m_new[:])
                    # context partials: part[e] = sum_t encT[e, t] * w[t]
                    part = smallp.tile([P, EK], F32, tag="part")
                    for ec in range(EK):
                        scr = smallp.tile([P, TC], F32, tag="scr")
                        nc.vector.affine_mul_reduce(
                            out=scr[:],
                            accum_out=part[:, ec : ec + 1],
                            in0=enc_t[:, ec, :].bitcast(F32),
                            in1=attn_rep[:, ts],
                            scale=1.0,
                            bias=0.0,
                        )
                    ctx_new = chainp.tile([P, EK], F32, tag="ctx")
                    nc.vector.tensor_scalar_mul(ctx_new[:], ctx_run[:], r_fac[:])
                    nc.vector.tensor_add(ctx_new[:], ctx_new[:], part[:])
                    m_run, z_run, ctx_run = m_new, z_new, ctx_new

                # ---- batch epilogue ---------------------------------------
                zinv = smallp.tile([P, 1], F32, tag="zinv")
                nc.vector.reciprocal(zinv[:], z_run[:])
                neg_mf = smallp.tile([P, 1], F32, tag="negmf")
                nc.vector.tensor_scalar_mul(neg_mf[:], m_run[:], -1.0)
                fchunk = smallp.tile([P, NTC], F32, tag="fchunk")
                nc.scalar.activation(
                    fchunk[:], m_hist[:], AF.Exp, bias=neg_mf[:], scale=1.0
                )
                nc.vector.tensor_scalar_mul(fchunk[:], fchunk[:], zinv[:])
                for tci in range(NTC):
                    ts = slice(tci * TC, (tci + 1) * TC)
                    nc.vector.tensor_scalar_mul(
                        attn_rep[:, ts], attn_rep[:, ts], fchunk[:, tci : tci + 1]
                    )
                nc.sync.dma_start(attn_out[b : b + 1, :], attn_rep[0:1, :])
                ctx_fin = smallp.tile([P, EK], F32, tag="ctxfin")
                nc.vector.tensor_scalar_mul(ctx_fin[:], ctx_run[:], zinv[:])
                nc.sync.dma_start(
                    ctx_out[b].rearrange("(ec p) -> p ec", p=P), ctx_fin[:]
                )

    nc.compile()
    return nc


def _get_nc():
    if "nc" not in _CACHE:
        _CACHE["nc"] = _build_nc()
    return _CACHE["nc"]


def _make_in_maps(encoder_feature, decoder_state, attention_mask, Wq, bq, Wk, bk):
    enc = np.asarray(encoder_feature, dtype=np.float32)
    decs = np.asarray(decoder_state, dtype=np.float32)
    msk = np.asarray(attention_mask, dtype=np.float32).reshape(B, T)
    wkT = np.ascontiguousarray(np.asarray(Wk, dtype=np.float32).T)
    wqT = np.ascontiguousarray(np.asarray(Wq, dtype=np.float32).T)
    bkv = np.ascontiguousarray(np.asarray(bk, dtype=np.float32))
    bqv = np.ascontiguousarray(np.asarray(bq, dtype=np.float32))
    in_maps = []
    for c in range(NCORES):
        sl = slice(c * BPC, (c + 1) * BPC)
        in_maps.append(
            {
                "enct": np.ascontiguousarray(enc[sl].transpose(0, 2, 1)),
                "wkt": wkT,
                "wqt": wqT,
                "dec": np.ascontiguousarray(decs[sl].T),
                "mask": np.ascontiguousarray(msk[sl]),
                "bk": bkv,
                "bq": bqv,
            }
        )
    return in_maps


def kernel(encoder_feature, decoder_state, attention_mask, Wq, bq, Wk, bk):
    from concourse.bass_utils import run_bass_kernel_spmd

    nc = _get_nc()
    in_maps = _make_in_maps(
        encoder_feature, decoder_state, attention_mask, Wq, bq, Wk, bk
    )
    res = run_bass_kernel_spmd(nc, in_maps, list(range(NCORES)))
    attn = np.concatenate([r["attn"] for r in res.results], axis=0)
    ctx = np.concatenate([r["ctx"] for r in res.results], axis=0)
    return attn.reshape(B, 1, T).astype(np.float32), ctx.astype(np.float32)
